# revision 31
# baseline (speedup 1.0000x reference)
"""Trainium2 Bass kernel for ManualCausalSelfAttention.

Full (unsharded) inputs -> full output. Internally shards across 8
NeuronCores: data-parallel over batch (4) x tensor-parallel over head
groups of 8 heads (2). Each core computes a partial output projection
(its 512 rows of W_proj); the host sums the two partials per batch.

Design notes (vs the 643 us v1 baseline; now ~364 us):
- single fused phase 1: x is streamed once per core (bf16); per token
  block the q/k projection groups interleave with v projection groups
  so the PE keeps streaming while DVE works through RoPE
  (stt/stt/shuffle on DVE, final add on GpSimd, V written bf16 with
  pad-zeroing by a DVE tensor_scalar).
- all matmul operands are bf16 (x, w_qk/w_v/w_proj, q'/k' store, P, V,
  y); PSUM accumulation stays fp32. Relative error ~7.6e-3.
- padding is folded into V (padded key rows of V and of the appended
  denominator column are zeroed) instead of an exp bias, which lets
  scores/exp/PV all trim to the causal-valid query range. The remaining
  per-diagonal-tile causal boundary is one static 128x128 bf16 triangle
  mask multiplied in on DVE (faster than gpsimd affine_select).
- softmax denominators: DVE copies the PSUM ones-row out and runs
  reciprocal_approx_fast (custom DVE op; needs lower_extended_insts and
  cannot read PSUM), then one DRAM hop (sync-queue DMAs) broadcasts the
  reciprocals across partitions. A PE rank-1 broadcast was tried and
  rejected: the normalize multiply would need two PSUM operands.
- phase 2 is software-pipelined over flat (query-block, pair) units in
  block order (1, 0, 2, 3): each unit's first score matmul is emitted
  inside the previous unit's last kt step (ACT rolls straight into the
  next exp), and each block's out-projection is emitted in 3-group
  chunks at the next block's pair boundaries to hide the denominator
  chains.
- PSUM: st tag [128,2,512]x2 shared by score tiles and out-proj tiles,
  pv tag [128,2,512]x2 for the PV accumulators (8 banks total).
- initial DMAs are spread across the sync/gpsimd/scalar queues so x,
  cos/sin and the first w_qk half land before the PE needs them.
"""

import functools
import os
import sys

import numpy as np

for _p in (
    "/root/.axon_site",
    "/root/.axon_site/_ro/trn_rl_repo",
    "/root/.axon_site/_ro/pypackages",
    "/opt/trn_rl_repo",
):
    if os.path.isdir(_p) and _p not in sys.path:
        sys.path.append(_p)

import concourse.bass as bass
import concourse.mybir as mybir
import concourse.tile as tile
from concourse.bass_utils import run_bass_kernel_spmd

# The PJRT boundary swallows python exceptions raised by the bass compile
# hook ("CallFunctionObjArgs: error condition !(py_result)"); wrap the hook
# so the real compile error is printed before being re-raised.
from concourse import bass2jax as _b2j

if not getattr(_b2j, "_hook_wrapped", False):
    _orig_hook = _b2j.neuronx_cc_hook

    def _loud_hook(*a, **k):
        try:
            return _orig_hook(*a, **k)
        except BaseException:
            import traceback

            traceback.print_exc()
            raise

    _b2j.neuronx_cc_hook = _loud_hook
    _b2j._hook_wrapped = True

HID = 1024
NH_GLOBAL = 16
NHL = 8  # heads per core
D = 64
THETA = 10000.0
F32 = mybir.dt.float32
F32R = mybir.dt.float32r
BF16 = mybir.dt.bfloat16
ALU = mybir.AluOpType
AFT = mybir.ActivationFunctionType

TB = 512  # token block (phase 1, matmul N)
QB = 512  # query block (phase 2)

# stream_shuffle mask swapping adjacent partition pairs (per 32-group)
PAIR_SWAP_MASK = [i + 1 - 2 * (i % 2) for i in range(32)]


def _split_sync_waits(nc, max_waits=1):
    """walrus in this environment rejects instructions carrying more than
    `max_waits` sem waits ("Too many sync wait commands"); Tile's kernel-tail
    drain carries one wait per logical proc. Split the excess waits onto
    preceding same-engine NOPs."""
    for fn in nc.m.functions:
        for blk in fn.blocks:
            if not any(
                ins.sync_info is not None
                and ins.sync_info.on_wait
                and len(ins.sync_info.on_wait)
                > (0 if isinstance(ins, mybir.InstDrain) else max_waits)
                for ins in blk.instructions
            ):
                continue
            new_insts = []
            for ins in blk.instructions:
                si = ins.sync_info
                limit = 0 if isinstance(ins, mybir.InstDrain) else max_waits
                if si is not None and si.on_wait and len(si.on_wait) > limit:
                    waits = list(si.on_wait)
                    if limit:
                        extra, keep = waits[:-limit], waits[-limit:]
                    else:
                        extra, keep = waits, []
                    for i in range(0, len(extra), max_waits):
                        new_insts.append(
                            mybir.InstNoOp(
                                name=f"{ins.name}-sw{i}",
                                sync_info=mybir.SyncInfo(
                                    on_wait=extra[i : i + max_waits], on_update=[]
                                ),
                                bass_nofuse=True,
                                engine=ins.engine,
                            )
                        )
                    ins.sync_info = mybir.SyncInfo(
                        on_wait=keep, on_update=list(si.on_update)
                    )
                new_insts.append(ins)
            blk.instructions[:] = new_insts


def build_kernel(s=2048, split_waits=True):
    """Build the per-core Bass module (same program on all 8 cores)."""
    nc = bass.Bass()
    nb = s // TB      # token blocks
    nqb = s // QB     # query blocks
    nkt = s // 128    # 128-wide key tiles / token tiles
    hkt = HID // 128  # hidden contraction tiles
    dqb = QB // 128   # 128-tiles per query block

    xT = nc.dram_tensor("xt", [HID, s], BF16, kind="ExternalInput")
    w_qk = nc.dram_tensor("w_qk", [HID, 2 * NHL * D], BF16, kind="ExternalInput")
    w_v = nc.dram_tensor("w_v", [HID, NHL * D], BF16, kind="ExternalInput")
    w_pr = nc.dram_tensor("w_proj", [NHL * D, HID], BF16, kind="ExternalInput")
    cos_d = nc.dram_tensor("cos_t", [128, s], F32, kind="ExternalInput")
    sin_d = nc.dram_tensor("sin_t", [128, s], F32, kind="ExternalInput")
    bqk_d = nc.dram_tensor("b_qk", [128, 8], F32, kind="ExternalInput")
    bv_d = nc.dram_tensor("b_v", [1, NHL * D], BF16, kind="ExternalInput")
    bpr_d = nc.dram_tensor("b_proj", [1, HID], BF16, kind="ExternalInput")
    padm_d = nc.dram_tensor("pad_mul", [128, nkt], F32, kind="ExternalInput")
    pvcol_d = nc.dram_tensor("pad_vcol", [128, nkt * NHL], BF16, kind="ExternalInput")
    onesb_d = nc.dram_tensor("ones_b", [1, 128], BF16, kind="ExternalInput")
    tri_d = nc.dram_tensor("tri_m", [128, 2 * 128], BF16, kind="ExternalInput")
    z_d = nc.dram_tensor("z", [s, HID], F32, kind="ExternalOutput")
    # scratch used to broadcast softmax reciprocals across partitions
    rcp_scr = nc.dram_tensor("rcp_scr", [nqb * 4, 2 * QB], F32)

    with tile.TileContext(nc) as tc:
        with tc.tile_pool(name="persist", bufs=1) as pp:
            # ---- persistent tiles; weight DMAs issued up front ----
            w_pr_sb = pp.tile([128, NHL * D // 128, HID], BF16, tag="wpr")
            bpr_sb = pp.tile([1, HID], BF16, tag="bpr")
            onesb_sb = pp.tile([1, 128], BF16, tag="onesb")
            tri_sb = pp.tile([128, 2, 128], BF16, tag="tri")
            padm_sb = pp.tile([128, nkt], F32, tag="padm")


            # q'T / k'T (RoPE'd, feature-major): 8 tiles of [128, s];
            # tiles 0..3 = Q head-pairs, 4..7 = K head-pairs.
            qk_store = [
                pp.tile([128, s], BF16, name=f"qk{mt}", tag=f"qk{mt}")
                for mt in range(8)
            ]
            # V (bf16) with denominator column (pad mask) appended per head;
            # inner dim padded to 66 for 4B alignment.
            v_sb = pp.tile([128, nkt, NHL, 66], BF16, tag="v")


            # ---------------- Phase 1: projections + RoPE ----------------
            with (
                tc.tile_pool(name="c1", bufs=1) as c1pool,
                tc.tile_pool(name="xin", bufs=2) as xpool,
                tc.tile_pool(name="rope", bufs=3) as rpool,
                tc.tile_pool(name="ps1", bufs=1, space="PSUM") as ps1,
            ):
                w_qk_sb = c1pool.tile([128, hkt, 2 * NHL * D], BF16, tag="wqk")
                w_v_sb = c1pool.tile([128, hkt, NHL * D], BF16, tag="wv")
                cos_sb = c1pool.tile([128, s], F32, tag="cos")
                sin_sb = c1pool.tile([128, s], F32, tag="sin")
                bqk_sb = c1pool.tile([128, 8], F32, tag="bqk")
                bv_sb = c1pool.tile([1, NHL * D], BF16, tag="bv")
                nc.gpsimd.dma_start(
                    w_qk_sb[:, :, 0:256],
                    w_qk.rearrange("(kt p) f -> p kt f", p=128)[:, :, 0:256],
                )
                nc.gpsimd.dma_start(
                    w_qk_sb[:, :, 256 : NHL * D],
                    w_qk.rearrange("(kt p) f -> p kt f", p=128)[
                        :, :, 256 : NHL * D
                    ],
                )
                nc.scalar.dma_start(
                    w_qk_sb[:, :, NHL * D : 2 * NHL * D],
                    w_qk.rearrange("(kt p) f -> p kt f", p=128)[
                        :, :, NHL * D : 2 * NHL * D
                    ],
                )
                nc.gpsimd.dma_start(
                    w_v_sb[:], w_v.rearrange("(kt p) f -> p kt f", p=128)
                )
                # phase-2-only loads ride behind the w_qk half on scalar
                nc.scalar.dma_start(
                    w_pr_sb[:], w_pr.rearrange("(sl p) f -> p sl f", p=128)
                )
                nc.scalar.dma_start(bpr_sb[:], bpr_d[:])
                nc.scalar.dma_start(
                    tri_sb.rearrange("p h q -> p (h q)"), tri_d[:]
                )
                nc.scalar.dma_start(
                    v_sb[:, :, :, 64:65],
                    pvcol_d.rearrange("p (k h o) -> p k h o", h=NHL, o=1),
                )
                # first x block + RoPE tables go first on the sync queue so
                # the PE and DVE can start at ~8us; everything phase-2-only
                # rides the scalar queue
                xt0 = xpool.tile([128, hkt, TB], BF16, tag="xt", name="xt0")
                for kh in range(2):
                    nc.sync.dma_start(
                        xt0[:, kh * 4 : (kh + 1) * 4, :],
                        xT.rearrange("(kt p) t -> p kt t", p=128)[
                            :, kh * 4 : (kh + 1) * 4, 0:TB
                        ],
                    )
                nc.sync.dma_start(onesb_sb[:], onesb_d[:])
                nc.sync.dma_start(padm_sb[:], padm_d[:])
                nc.sync.dma_start(cos_sb[:], cos_d[:])
                nc.sync.dma_start(sin_sb[:], sin_d[:])
                nc.sync.dma_start(bqk_sb[:], bqk_d[:])
                nc.sync.dma_start(bv_sb[:], bv_d[:])
                for nbi in range(nb):
                    tsl = slice(nbi * TB, (nbi + 1) * TB)
                    if nbi == 0:
                        xt = xt0
                    else:
                        xt = xpool.tile([128, hkt, TB], BF16, tag="xt")
                        nc.sync.dma_start(
                            xt[:],
                            xT.rearrange("(kt p) t -> p kt t", p=128)[:, :, tsl],
                        )
                    def v_group(vt):
                        psv = ps1.tile([128, NHL * D], F32, tag="v", bufs=4)
                        for kt in range(hkt):
                            nc.tensor.matmul(
                                psv[:],
                                xt[:, kt, vt * 128 : (vt + 1) * 128],
                                w_v_sb[:, kt, :],
                                start=(kt == 0),
                                stop=False,
                            )
                        nc.tensor.matmul(
                            psv[:], onesb_sb[:], bv_sb[:], start=False, stop=True
                        )
                        ktix = nbi * (TB // 128) + vt
                        # write V bf16, zeroing padded key rows (per-token
                        # pad multiplier as the tensor_scalar operand)
                        nc.vector.tensor_scalar(
                            out=v_sb[:, ktix, :, 0:D],
                            in0=psv.rearrange("p (h d) -> p h d", d=D),
                            scalar1=padm_sb[:, ktix : ktix + 1],
                            scalar2=None,
                            op0=ALU.mult,
                        )

                    for mt in range(8):
                        ps = ps1.tile([128, TB], F32, tag="qk", bufs=4)
                        for kt in range(hkt):
                            nc.tensor.matmul(
                                ps[:],
                                w_qk_sb[:, kt, mt * 128 : (mt + 1) * 128],
                                xt[:, kt, :],
                                start=(kt == 0),
                                stop=(kt == hkt - 1),
                            )
                        # RoPE on DVE (PSUM reads); final add on GpSimd (SBUF)
                        t1 = rpool.tile([128, TB], F32, tag="t1")
                        nc.vector.scalar_tensor_tensor(
                            out=t1[:],
                            in0=ps[:],
                            scalar=bqk_sb[:, mt : mt + 1],
                            in1=cos_sb[:, tsl],
                            op0=ALU.add,
                            op1=ALU.mult,
                        )
                        t2p = rpool.tile([128, TB], F32, tag="t2p")
                        nc.vector.scalar_tensor_tensor(
                            out=t2p[:],
                            in0=ps[:],
                            scalar=bqk_sb[:, mt : mt + 1],
                            in1=sin_sb[:, tsl],
                            op0=ALU.add,
                            op1=ALU.mult,
                        )
                        t2 = rpool.tile([128, TB], F32, tag="t2")
                        nc.vector.stream_shuffle(t2[:], t2p[:], PAIR_SWAP_MASK)
                        nc.gpsimd.tensor_add(qk_store[mt][:, tsl], t1[:], t2[:])
                        if mt % 2 == 1:
                            # interleave a V group (no DVE dependency) so the
                            # PE stays busy while DVE works through RoPE
                            v_group(mt // 2)

            # ---------------- Phase 2: attention + out-proj ----------------
            with (
                tc.tile_pool(name="pt", bufs=4) as ptpool,
                tc.tile_pool(name="sm", bufs=3) as smpool,
                tc.tile_pool(name="ytn", bufs=9) as ytnpool,
                tc.tile_pool(name="zs", bufs=4) as zspool,
                tc.tile_pool(name="ps2", bufs=1, space="PSUM") as ps2,
            ):
                def outproj_groups(oqb, oytns):
                    def one(tt, ob):
                        def emit():
                            gt = oqb * dqb + tt
                            zp = ps2.tile(
                                [128, 2, 512], F32, tag="st", bufs=2, name="zp"
                            )
                            for pair in range(4):
                                nc.tensor.matmul(
                                    zp[:, 0, :],
                                    oytns[pair][:, tt * 128 : (tt + 1) * 128],
                                    w_pr_sb[:, pair, ob * 512 : (ob + 1) * 512],
                                    start=(pair == 0),
                                    stop=False,
                                )
                            nc.tensor.matmul(
                                zp[:, 0, :],
                                onesb_sb[:],
                                bpr_sb[:, ob * 512 : (ob + 1) * 512],
                                start=False,
                                stop=True,
                            )
                            zs = zspool.tile([128, 512], F32, tag="zs")
                            nc.vector.tensor_scalar(
                                out=zs[:],
                                in0=zp[:, 0, :],
                                scalar1=padm_sb[:, gt : gt + 1],
                                scalar2=None,
                                op0=ALU.mult,
                            )
                            nc.gpsimd.dma_start(
                                z_d[
                                    gt * 128 : (gt + 1) * 128,
                                    ob * 512 : (ob + 1) * 512,
                                ],
                                zs[:],
                            )

                        return emit

                    return [
                        one(tt, ob)
                        for tt in range(dqb)
                        for ob in range(HID // 512)
                    ]

                # start mid-depth, slot the shallow block second (its exposed
                # denominator chains hide under the next block), end deepest
                def make_st(qb, pair):
                    qst = qk_store[pair]
                    kst = qk_store[4 + pair]
                    qoff = qb * QB

                    def do_st(kt):
                        q0 = max(0, kt * 128 - qoff)
                        stp = ps2.tile(
                            [128, 2, QB], F32, tag="st", bufs=2, name="stp"
                        )
                        for h2 in (0, 1):
                            lo = h2 * 64
                            nc.tensor.matmul(
                                stp[:, h2, q0:QB],
                                kst[lo : lo + 64, kt * 128 : (kt + 1) * 128],
                                qst[lo : lo + 64, qoff + q0 : qoff + QB],
                                start=True,
                                stop=True,
                                tile_position=(lo, 0),
                            )
                        return stp

                    return do_st

                # flat (qb, pair) unit list, software-pipelined: the next
                # unit's st(0) is emitted inside this unit's last kt step so
                # ACT rolls straight into the next pair's first exp
                units = [(qb, pair) for qb in (1, 0, 2, 3) for pair in range(4)]
                pending = []
                ytns = []
                carry = make_st(*units[0])(0)
                for ui, (qb, pair) in enumerate(units):
                    nkts = (qb + 1) * dqb
                    qoff = qb * QB
                    do_st = make_st(qb, pair)
                    yps = ps2.tile(
                        [128, 2, QB], F32, tag="pv", bufs=2, name="pv"
                    )
                    prev = carry
                    for kt in range(nkts):
                        if kt + 1 < nkts:
                            nxt = do_st(kt + 1)
                        else:
                            nxt = None
                            if ui + 1 < len(units):
                                carry = make_st(*units[ui + 1])(0)
                        q0 = max(0, kt * 128 - qoff)
                        pt = ptpool.tile([128, 2, QB], BF16, tag="pt")
                        nc.scalar.activation(
                            pt[:, :, q0:QB],
                            prev[:, :, q0:QB],
                            AFT.Exp,
                            scale=float(D) ** -0.5,
                        )
                        if kt >= nkts - dqb:
                            # diagonal tile: zero P where q < k in the single
                            # ambiguous 128-col block. The boundary triangle
                            # is identical for every diagonal tile, so it is
                            # one static bf16 mask multiplied in on DVE
                            # (faster than gpsimd affine_select, and off the
                            # exp->mask->PV critical path's slow engine)
                            ap = pt[:, :, q0 : q0 + 128]
                            nc.vector.tensor_mul(ap, ap, tri_sb[:, :, :])
                        for h2 in (0, 1):
                            head = pair * 2 + h2
                            nc.tensor.matmul(
                                yps[0 : D + 1, h2, q0:QB],
                                v_sb[:, kt, head, 0 : D + 1],
                                pt[:, h2, q0:QB],
                                start=(kt == 0),
                                stop=(kt == nkts - 1),
                            )
                        prev = nxt
                    # softmax denominators: copy + fast reciprocal on DVE
                    # (off ACT so the next pair's exp isn't queued behind
                    # it), one DRAM hop to broadcast across partitions
                    row = qb * 4 + pair
                    den = smpool.tile([1, 2 * QB], F32, tag="den")
                    nc.vector.tensor_copy(
                        den[:],
                        yps[D : D + 1, :, :].rearrange("p h q -> p (h q)"),
                    )
                    rcp = smpool.tile([1, 2, QB], F32, tag="rcp")
                    nc.vector.reciprocal_approx_fast(
                        rcp.rearrange("o h q -> o (h q)"), den[:]
                    )
                    nc.sync.dma_start(
                        rcp_scr[row : row + 1, :],
                        rcp.rearrange("o h q -> o (h q)"),
                    )
                    rb = smpool.tile([64, 2, QB], F32, tag="rb")
                    nc.sync.dma_start(
                        rb.rearrange("p h q -> p (h q)"),
                        rcp_scr[row : row + 1, :].broadcast_to([64, 2 * QB]),
                    )
                    ytn = ytnpool.tile([128, QB], BF16, tag="ytn")
                    for h2 in (0, 1):
                        nc.vector.tensor_mul(
                            ytn[h2 * 64 : (h2 + 1) * 64, :],
                            yps[0:D, h2, :],
                            rb[:, h2, :],
                        )
                    ytns.append(ytn)
                    # previous query block's out-proj, in chunks at pair
                    # boundaries: PE work that hides this pair's
                    # denominator chain without disrupting the st ring
                    for _ in range(3):
                        if pending:
                            pending.pop(0)()
                    if pair == 3:
                        for g in pending:
                            g()
                        pending = outproj_groups(qb, ytns)
                        ytns = []
                for g in pending:
                    g()
    # populate .instr bytes for extended-inst ISA subclasses (custom DVE
    # ops); without this the NEFF compiler fails with "ISA wrong length"
    from concourse.library_overlay import lower_extended_insts

    lower_extended_insts(nc)
    if split_waits:
        _split_sync_waits(nc)
    return nc


@functools.lru_cache(maxsize=2)
def _built(s):
    return build_kernel(s)


def _rope_tables(s):
    j = np.arange(D // 2, dtype=np.float64)
    inv = THETA ** (-2.0 * j / D)
    ang = np.arange(s, dtype=np.float64)[:, None] * inv[None, :]  # [s, 32]
    cos = np.cos(ang).T  # [32, s]
    sin = np.sin(ang).T
    cos64 = np.repeat(cos, 2, axis=0)  # rows 2j, 2j+1 identical
    sin64 = np.repeat(sin, 2, axis=0)
    # "pre-swap" sign convention: the kernel multiplies by this table BEFORE
    # pair-swapping partitions, so odd rows carry the minus sign.
    sin64[1::2, :] *= -1.0
    cos128 = np.concatenate([cos64, cos64], axis=0).astype(np.float32)
    sin128 = np.concatenate([sin64, sin64], axis=0).astype(np.float32)
    return np.ascontiguousarray(cos128), np.ascontiguousarray(sin128)


def _col_tiled(vec, tile_rows=128):
    """[n] -> [tile_rows, n//tile_rows], column t = vec[t*128:(t+1)*128]."""
    n = vec.shape[0]
    return np.ascontiguousarray(vec.reshape(n // tile_rows, tile_rows).T)


def _bf16(a):
    import ml_dtypes

    return np.ascontiguousarray(np.asarray(a).astype(ml_dtypes.bfloat16))


def make_in_maps(x, attention_padding, W_qkv, b_qkv, W_proj, b_proj):
    x = np.asarray(x, dtype=np.float32)
    pad = np.asarray(attention_padding).astype(bool)
    W_qkv = np.asarray(W_qkv, dtype=np.float32)
    b_qkv = np.asarray(b_qkv, dtype=np.float32)
    W_proj = np.asarray(W_proj, dtype=np.float32)
    b_proj = np.asarray(b_proj, dtype=np.float32)
    B, s, _ = x.shape
    nkt = s // 128
    cos128, sin128 = _rope_tables(s)

    per_hp = {}
    for hp in range(2):
        hs = slice(hp * NHL * D, (hp + 1) * NHL * D)
        Wq = W_qkv[:, 0:HID][:, hs]
        Wk = W_qkv[:, HID : 2 * HID][:, hs]
        Wv = W_qkv[:, 2 * HID : 3 * HID][:, hs]
        bq = b_qkv[0:HID][hs]
        bk = b_qkv[HID : 2 * HID][hs]
        bv = b_qkv[2 * HID : 3 * HID][hs]
        bqk = np.concatenate([bq, bk])
        per_hp[hp] = dict(
            w_qk=_bf16(np.concatenate([Wq, Wk], axis=1)),
            w_v=_bf16(Wv),
            w_proj=_bf16(W_proj[hs, :]),
            b_qk=_col_tiled(bqk),
            b_v=_bf16(bv[None, :]),
            b_proj=_bf16((b_proj if hp == 0 else np.zeros_like(b_proj))[None, :]),
        )

    per_b = {}
    for b in range(B):
        p = pad[b].astype(np.float32)
        per_b[b] = dict(
            xt=_bf16(x[b].T),
            pad_mul=_col_tiled(p),
            pad_vcol=_bf16(np.repeat(_col_tiled(p), NHL, axis=1)),
        )

    in_maps = []
    for c in range(2 * B):
        b, hp = c // 2, c % 2
        m = dict(per_hp[hp])
        m.update(per_b[b])
        m["cos_t"] = cos128
        m["sin_t"] = sin128
        m["ones_b"] = _bf16(np.ones((1, 128)))
        j = np.arange(128)
        tri = (j[None, :] >= j[:, None]).astype(np.float32)  # keep q >= k
        m["tri_m"] = _bf16(np.tile(tri, (1, 2)))
        in_maps.append(m)
    return in_maps


def run(x, attention_padding, W_qkv, b_qkv, W_proj, b_proj, trace=False, **spmd_kw):
    x = np.asarray(x, dtype=np.float32)
    B, s, _ = x.shape
    nc = _built(s)
    in_maps = make_in_maps(x, attention_padding, W_qkv, b_qkv, W_proj, b_proj)
    res = run_bass_kernel_spmd(nc, in_maps, list(range(2 * B)), trace=trace, **spmd_kw)
    out = np.stack(
        [res.results[2 * b]["z"] + res.results[2 * b + 1]["z"] for b in range(B)]
    ).astype(np.float32)
    return out, res


def kernel(x, attention_padding, W_qkv, b_qkv, W_proj, b_proj, train=None, **_):
    out, _res = run(x, attention_padding, W_qkv, b_qkv, W_proj, b_proj)
    return out


# revision 33
# speedup vs baseline: 1.0176x; 1.0176x over previous
"""Trainium2 Bass kernel for ManualCausalSelfAttention.

Full (unsharded) inputs -> full output. Internally shards across 8
NeuronCores: data-parallel over batch (4) x tensor-parallel over head
groups of 8 heads (2). Each core computes a partial output projection
(its 512 rows of W_proj); the host sums the two partials per batch.

Design notes (vs the 643 us v1 baseline; now ~364 us):
- single fused phase 1: x is streamed once per core (bf16); per token
  block the q/k projection groups interleave with v projection groups
  so the PE keeps streaming while DVE works through RoPE
  (stt/stt/shuffle on DVE, final add on GpSimd, V written bf16 with
  pad-zeroing by a DVE tensor_scalar).
- all matmul operands are bf16 (x, w_qk/w_v/w_proj, q'/k' store, P, V,
  y); PSUM accumulation stays fp32. Relative error ~7.6e-3.
- padding is folded into V (padded key rows of V and of the appended
  denominator column are zeroed) instead of an exp bias, which lets
  scores/exp/PV all trim to the causal-valid query range. The remaining
  per-diagonal-tile causal boundary is one static 128x128 bf16 triangle
  mask multiplied in on DVE (faster than gpsimd affine_select).
- softmax denominators: DVE copies the PSUM ones-row out and runs
  reciprocal_approx_fast (custom DVE op; needs lower_extended_insts and
  cannot read PSUM), then one DRAM hop (sync-queue DMAs) broadcasts the
  reciprocals across partitions. A PE rank-1 broadcast was tried and
  rejected: the normalize multiply would need two PSUM operands.
- phase 2 is software-pipelined over flat (query-block, pair) units in
  block order (1, 0, 2, 3): each unit's first score matmul is emitted
  inside the previous unit's last kt step (ACT rolls straight into the
  next exp), and each block's out-projection is emitted in 3-group
  chunks at the next block's pair boundaries to hide the denominator
  chains.
- PSUM: st tag [128,2,512]x2 shared by score tiles and out-proj tiles,
  pv tag [128,2,512]x2 for the PV accumulators (8 banks total).
- initial DMAs are spread across the sync/gpsimd/scalar queues so x,
  cos/sin and the first w_qk half land before the PE needs them.
"""

import functools
import os
import sys

import numpy as np

for _p in (
    "/root/.axon_site",
    "/root/.axon_site/_ro/trn_rl_repo",
    "/root/.axon_site/_ro/pypackages",
    "/opt/trn_rl_repo",
):
    if os.path.isdir(_p) and _p not in sys.path:
        sys.path.append(_p)

import concourse.bass as bass
import concourse.mybir as mybir
import concourse.tile as tile
from concourse.bass_utils import run_bass_kernel_spmd

# The PJRT boundary swallows python exceptions raised by the bass compile
# hook ("CallFunctionObjArgs: error condition !(py_result)"); wrap the hook
# so the real compile error is printed before being re-raised.
from concourse import bass2jax as _b2j

if not getattr(_b2j, "_hook_wrapped", False):
    _orig_hook = _b2j.neuronx_cc_hook

    def _loud_hook(*a, **k):
        try:
            return _orig_hook(*a, **k)
        except BaseException:
            import traceback

            traceback.print_exc()
            raise

    _b2j.neuronx_cc_hook = _loud_hook
    _b2j._hook_wrapped = True

HID = 1024
NH_GLOBAL = 16
NHL = 8  # heads per core
D = 64
THETA = 10000.0
F32 = mybir.dt.float32
F32R = mybir.dt.float32r
BF16 = mybir.dt.bfloat16
ALU = mybir.AluOpType
AFT = mybir.ActivationFunctionType

TB = 512  # token block (phase 1, matmul N)
QB = 512  # query block (phase 2)

# stream_shuffle mask swapping adjacent partition pairs (per 32-group)
PAIR_SWAP_MASK = [i + 1 - 2 * (i % 2) for i in range(32)]


def _split_sync_waits(nc, max_waits=1):
    """walrus in this environment rejects instructions carrying more than
    `max_waits` sem waits ("Too many sync wait commands"); Tile's kernel-tail
    drain carries one wait per logical proc. Split the excess waits onto
    preceding same-engine NOPs."""
    for fn in nc.m.functions:
        for blk in fn.blocks:
            if not any(
                ins.sync_info is not None
                and ins.sync_info.on_wait
                and len(ins.sync_info.on_wait)
                > (0 if isinstance(ins, mybir.InstDrain) else max_waits)
                for ins in blk.instructions
            ):
                continue
            new_insts = []
            for ins in blk.instructions:
                si = ins.sync_info
                limit = 0 if isinstance(ins, mybir.InstDrain) else max_waits
                if si is not None and si.on_wait and len(si.on_wait) > limit:
                    waits = list(si.on_wait)
                    if limit:
                        extra, keep = waits[:-limit], waits[-limit:]
                    else:
                        extra, keep = waits, []
                    for i in range(0, len(extra), max_waits):
                        new_insts.append(
                            mybir.InstNoOp(
                                name=f"{ins.name}-sw{i}",
                                sync_info=mybir.SyncInfo(
                                    on_wait=extra[i : i + max_waits], on_update=[]
                                ),
                                bass_nofuse=True,
                                engine=ins.engine,
                            )
                        )
                    ins.sync_info = mybir.SyncInfo(
                        on_wait=keep, on_update=list(si.on_update)
                    )
                new_insts.append(ins)
            blk.instructions[:] = new_insts


def build_kernel(s=2048, split_waits=True):
    """Build the per-core Bass module (same program on all 8 cores)."""
    nc = bass.Bass()
    nb = s // TB      # token blocks
    nqb = s // QB     # query blocks
    nkt = s // 128    # 128-wide key tiles / token tiles
    hkt = HID // 128  # hidden contraction tiles
    dqb = QB // 128   # 128-tiles per query block

    xT = nc.dram_tensor("xt", [HID, s], BF16, kind="ExternalInput")
    w_qk = nc.dram_tensor("w_qk", [HID, 2 * NHL * D], BF16, kind="ExternalInput")
    w_v = nc.dram_tensor("w_v", [HID, NHL * D], BF16, kind="ExternalInput")
    w_pr = nc.dram_tensor("w_proj", [NHL * D, HID], BF16, kind="ExternalInput")
    cos_d = nc.dram_tensor("cos_t", [128, s], F32, kind="ExternalInput")
    sin_d = nc.dram_tensor("sin_t", [128, s], F32, kind="ExternalInput")
    bqk_d = nc.dram_tensor("b_qk", [128, 8], F32, kind="ExternalInput")
    bv_d = nc.dram_tensor("b_v", [1, NHL * D], BF16, kind="ExternalInput")
    bpr_d = nc.dram_tensor("b_proj", [1, HID], BF16, kind="ExternalInput")
    padm_d = nc.dram_tensor("pad_mul", [128, nkt], F32, kind="ExternalInput")
    pvcol_d = nc.dram_tensor("pad_vcol", [128, nkt * NHL], BF16, kind="ExternalInput")
    onesb_d = nc.dram_tensor("ones_b", [1, 128], BF16, kind="ExternalInput")
    tri_d = nc.dram_tensor("tri_m", [128, 2 * 128], BF16, kind="ExternalInput")
    z_d = nc.dram_tensor("z", [s, HID], F32, kind="ExternalOutput")
    # scratch used to broadcast softmax reciprocals across partitions
    rcp_scr = nc.dram_tensor("rcp_scr", [nqb * 4, 2 * QB], F32)

    with tile.TileContext(nc) as tc:
        with tc.tile_pool(name="persist", bufs=1) as pp:
            # ---- persistent tiles; weight DMAs issued up front ----
            w_pr_sb = pp.tile([128, NHL * D // 128, HID], BF16, tag="wpr")
            bpr_sb = pp.tile([1, HID], BF16, tag="bpr")
            onesb_sb = pp.tile([1, 128], BF16, tag="onesb")
            tri_sb = pp.tile([128, 2, 128], BF16, tag="tri")
            padm_sb = pp.tile([128, nkt], F32, tag="padm")


            # q'T / k'T (RoPE'd, feature-major): 8 tiles of [128, s];
            # tiles 0..3 = Q head-pairs, 4..7 = K head-pairs.
            qk_store = [
                pp.tile([128, s], BF16, name=f"qk{mt}", tag=f"qk{mt}")
                for mt in range(8)
            ]
            # V (bf16) with denominator column (pad mask) appended per head;
            # inner dim padded to 66 for 4B alignment.
            v_sb = pp.tile([128, nkt, NHL, 66], BF16, tag="v")


            # ---------------- Phase 1: projections + RoPE ----------------
            with (
                tc.tile_pool(name="c1", bufs=1) as c1pool,
                tc.tile_pool(name="xin", bufs=2) as xpool,
                tc.tile_pool(name="rope", bufs=3) as rpool,
                tc.tile_pool(name="ps1", bufs=1, space="PSUM") as ps1,
            ):
                w_qk_sb = c1pool.tile([128, hkt, 2 * NHL * D], BF16, tag="wqk")
                w_v_sb = c1pool.tile([128, hkt, NHL * D], BF16, tag="wv")
                cos_sb = c1pool.tile([128, s], F32, tag="cos")
                sin_sb = c1pool.tile([128, s], F32, tag="sin")
                bqk_sb = c1pool.tile([128, 8], F32, tag="bqk")
                bv_sb = c1pool.tile([1, NHL * D], BF16, tag="bv")
                nc.gpsimd.dma_start(
                    w_qk_sb[:, :, 0:256],
                    w_qk.rearrange("(kt p) f -> p kt f", p=128)[:, :, 0:256],
                )
                nc.gpsimd.dma_start(
                    w_qk_sb[:, :, 256 : NHL * D],
                    w_qk.rearrange("(kt p) f -> p kt f", p=128)[
                        :, :, 256 : NHL * D
                    ],
                )
                nc.scalar.dma_start(
                    w_qk_sb[:, :, NHL * D : 2 * NHL * D],
                    w_qk.rearrange("(kt p) f -> p kt f", p=128)[
                        :, :, NHL * D : 2 * NHL * D
                    ],
                )
                nc.gpsimd.dma_start(
                    w_v_sb[:], w_v.rearrange("(kt p) f -> p kt f", p=128)
                )
                # phase-2-only loads ride behind the w_qk half on scalar
                nc.scalar.dma_start(
                    w_pr_sb[:], w_pr.rearrange("(sl p) f -> p sl f", p=128)
                )
                nc.scalar.dma_start(bpr_sb[:], bpr_d[:])
                nc.scalar.dma_start(
                    tri_sb.rearrange("p h q -> p (h q)"), tri_d[:]
                )
                nc.scalar.dma_start(
                    v_sb[:, :, :, 64:65],
                    pvcol_d.rearrange("p (k h o) -> p k h o", h=NHL, o=1),
                )
                # first x block + RoPE tables go first on the sync queue so
                # the PE and DVE can start at ~8us; everything phase-2-only
                # rides the scalar queue
                xt0 = xpool.tile([128, hkt, TB], BF16, tag="xt", name="xt0")
                for kh in range(2):
                    nc.sync.dma_start(
                        xt0[:, kh * 4 : (kh + 1) * 4, :],
                        xT.rearrange("(kt p) t -> p kt t", p=128)[
                            :, kh * 4 : (kh + 1) * 4, 0:TB
                        ],
                    )
                nc.sync.dma_start(onesb_sb[:], onesb_d[:])
                nc.sync.dma_start(padm_sb[:], padm_d[:])
                nc.sync.dma_start(cos_sb[:], cos_d[:])
                nc.sync.dma_start(sin_sb[:], sin_d[:])
                nc.sync.dma_start(bqk_sb[:], bqk_d[:])
                nc.sync.dma_start(bv_sb[:], bv_d[:])
                for nbi in range(nb):
                    tsl = slice(nbi * TB, (nbi + 1) * TB)
                    if nbi == 0:
                        xt = xt0
                    else:
                        xt = xpool.tile([128, hkt, TB], BF16, tag="xt")
                        nc.sync.dma_start(
                            xt[:],
                            xT.rearrange("(kt p) t -> p kt t", p=128)[:, :, tsl],
                        )
                    def v_group(vt):
                        psv = ps1.tile([128, NHL * D], F32, tag="v", bufs=4)
                        for kt in range(hkt):
                            nc.tensor.matmul(
                                psv[:],
                                xt[:, kt, vt * 128 : (vt + 1) * 128],
                                w_v_sb[:, kt, :],
                                start=(kt == 0),
                                stop=False,
                            )
                        nc.tensor.matmul(
                            psv[:], onesb_sb[:], bv_sb[:], start=False, stop=True
                        )
                        ktix = nbi * (TB // 128) + vt
                        # write V bf16, zeroing padded key rows (per-token
                        # pad multiplier as the tensor_scalar operand)
                        nc.vector.tensor_scalar(
                            out=v_sb[:, ktix, :, 0:D],
                            in0=psv.rearrange("p (h d) -> p h d", d=D),
                            scalar1=padm_sb[:, ktix : ktix + 1],
                            scalar2=None,
                            op0=ALU.mult,
                        )

                    for mt in range(8):
                        ps = ps1.tile([128, TB], F32, tag="qk", bufs=4)
                        for kt in range(hkt):
                            nc.tensor.matmul(
                                ps[:],
                                w_qk_sb[:, kt, mt * 128 : (mt + 1) * 128],
                                xt[:, kt, :],
                                start=(kt == 0),
                                stop=(kt == hkt - 1),
                            )
                        # RoPE on DVE (PSUM reads); final add on GpSimd (SBUF)
                        t1 = rpool.tile([128, TB], F32, tag="t1")
                        nc.vector.scalar_tensor_tensor(
                            out=t1[:],
                            in0=ps[:],
                            scalar=bqk_sb[:, mt : mt + 1],
                            in1=cos_sb[:, tsl],
                            op0=ALU.add,
                            op1=ALU.mult,
                        )
                        t2p = rpool.tile([128, TB], F32, tag="t2p")
                        nc.vector.scalar_tensor_tensor(
                            out=t2p[:],
                            in0=ps[:],
                            scalar=bqk_sb[:, mt : mt + 1],
                            in1=sin_sb[:, tsl],
                            op0=ALU.add,
                            op1=ALU.mult,
                        )
                        t2 = rpool.tile([128, TB], F32, tag="t2")
                        nc.vector.stream_shuffle(t2[:], t2p[:], PAIR_SWAP_MASK)
                        nc.gpsimd.tensor_add(qk_store[mt][:, tsl], t1[:], t2[:])
                        if mt % 2 == 1:
                            # interleave a V group (no DVE dependency) so the
                            # PE stays busy while DVE works through RoPE
                            v_group(mt // 2)

            # ---------------- Phase 2: attention + out-proj ----------------
            with (
                tc.tile_pool(name="pt", bufs=4) as ptpool,
                tc.tile_pool(name="sm", bufs=3) as smpool,
                tc.tile_pool(name="ytn", bufs=9) as ytnpool,
                tc.tile_pool(name="zs", bufs=4) as zspool,
                tc.tile_pool(name="ps2", bufs=1, space="PSUM") as ps2,
            ):
                def outproj_groups(oqb, oytns):
                    def one(tt, ob):
                        def emit():
                            gt = oqb * dqb + tt
                            zp = ps2.tile(
                                [128, 2, 512], F32, tag="st", bufs=2, name="zp"
                            )
                            for pair in range(4):
                                nc.tensor.matmul(
                                    zp[:, 0, :],
                                    oytns[pair][:, tt * 128 : (tt + 1) * 128],
                                    w_pr_sb[:, pair, ob * 512 : (ob + 1) * 512],
                                    start=(pair == 0),
                                    stop=False,
                                )
                            nc.tensor.matmul(
                                zp[:, 0, :],
                                onesb_sb[:],
                                bpr_sb[:, ob * 512 : (ob + 1) * 512],
                                start=False,
                                stop=True,
                            )
                            zs = zspool.tile([128, 512], F32, tag="zs")
                            nc.vector.tensor_scalar(
                                out=zs[:],
                                in0=zp[:, 0, :],
                                scalar1=padm_sb[:, gt : gt + 1],
                                scalar2=None,
                                op0=ALU.mult,
                            )
                            nc.sync.dma_start(
                                z_d[
                                    gt * 128 : (gt + 1) * 128,
                                    ob * 512 : (ob + 1) * 512,
                                ],
                                zs[:],
                            )

                        return emit

                    return [
                        one(tt, ob)
                        for tt in range(dqb)
                        for ob in range(HID // 512)
                    ]

                # start mid-depth, slot the shallow block second (its exposed
                # denominator chains hide under the next block), end deepest
                def make_st(qb, pair):
                    qst = qk_store[pair]
                    kst = qk_store[4 + pair]
                    qoff = qb * QB

                    def do_st(kt):
                        q0 = max(0, kt * 128 - qoff)
                        stp = ps2.tile(
                            [128, 2, QB], F32, tag="st", bufs=2, name="stp"
                        )
                        for h2 in (0, 1):
                            lo = h2 * 64
                            nc.tensor.matmul(
                                stp[:, h2, q0:QB],
                                kst[lo : lo + 64, kt * 128 : (kt + 1) * 128],
                                qst[lo : lo + 64, qoff + q0 : qoff + QB],
                                start=True,
                                stop=True,
                                tile_position=(lo, 0),
                            )
                        return stp

                    return do_st

                # flat (qb, pair) unit list, software-pipelined: the next
                # unit's st(0) is emitted inside this unit's last kt step so
                # ACT rolls straight into the next pair's first exp
                units = [(qb, pair) for qb in (1, 0, 2, 3) for pair in range(4)]
                pending = []
                ytns = []
                carry = make_st(*units[0])(0)
                for ui, (qb, pair) in enumerate(units):
                    nkts = (qb + 1) * dqb
                    qoff = qb * QB
                    do_st = make_st(qb, pair)
                    yps = ps2.tile(
                        [128, 2, QB], F32, tag="pv", bufs=2, name="pv"
                    )
                    prev = carry
                    for kt in range(nkts):
                        if kt + 1 < nkts:
                            nxt = do_st(kt + 1)
                        else:
                            nxt = None
                            if ui + 1 < len(units):
                                carry = make_st(*units[ui + 1])(0)
                        q0 = max(0, kt * 128 - qoff)
                        pt = ptpool.tile([128, 2, QB], BF16, tag="pt")
                        nc.scalar.activation(
                            pt[:, :, q0:QB],
                            prev[:, :, q0:QB],
                            AFT.Exp,
                            scale=float(D) ** -0.5,
                        )
                        if kt >= nkts - dqb:
                            # diagonal tile: zero P where q < k in the single
                            # ambiguous 128-col block. The boundary triangle
                            # is identical for every diagonal tile, so it is
                            # one static bf16 mask multiplied in on DVE
                            # (faster than gpsimd affine_select, and off the
                            # exp->mask->PV critical path's slow engine)
                            ap = pt[:, :, q0 : q0 + 128]
                            nc.vector.tensor_mul(ap, ap, tri_sb[:, :, :])
                        for h2 in (0, 1):
                            head = pair * 2 + h2
                            nc.tensor.matmul(
                                yps[0 : D + 1, h2, q0:QB],
                                v_sb[:, kt, head, 0 : D + 1],
                                pt[:, h2, q0:QB],
                                start=(kt == 0),
                                stop=(kt == nkts - 1),
                            )
                        prev = nxt
                    # softmax denominators: copy + fast reciprocal on DVE
                    # (off ACT so the next pair's exp isn't queued behind
                    # it), one DRAM hop to broadcast across partitions
                    row = qb * 4 + pair
                    den = smpool.tile([1, 2 * QB], F32, tag="den")
                    nc.vector.tensor_copy(
                        den[:],
                        yps[D : D + 1, :, :].rearrange("p h q -> p (h q)"),
                    )
                    rcp = smpool.tile([1, 2, QB], F32, tag="rcp")
                    nc.vector.reciprocal_approx_fast(
                        rcp.rearrange("o h q -> o (h q)"), den[:]
                    )
                    nc.sync.dma_start(
                        rcp_scr[row : row + 1, :],
                        rcp.rearrange("o h q -> o (h q)"),
                    )
                    rb = smpool.tile([64, 2, QB], F32, tag="rb")
                    nc.sync.dma_start(
                        rb.rearrange("p h q -> p (h q)"),
                        rcp_scr[row : row + 1, :].broadcast_to([64, 2 * QB]),
                    )
                    ytn = ytnpool.tile([128, QB], BF16, tag="ytn")
                    for h2 in (0, 1):
                        nc.vector.tensor_mul(
                            ytn[h2 * 64 : (h2 + 1) * 64, :],
                            yps[0:D, h2, :],
                            rb[:, h2, :],
                        )
                    ytns.append(ytn)
                    # previous query block's out-proj, in chunks at pair
                    # boundaries: PE work that hides this pair's
                    # denominator chain without disrupting the st ring
                    for _ in range(4):
                        if pending:
                            pending.pop(0)()
                    if pair == 3:
                        for g in pending:
                            g()
                        pending = outproj_groups(qb, ytns)
                        ytns = []
                for g in pending:
                    g()
    # populate .instr bytes for extended-inst ISA subclasses (custom DVE
    # ops); without this the NEFF compiler fails with "ISA wrong length"
    from concourse.library_overlay import lower_extended_insts

    lower_extended_insts(nc)
    if split_waits:
        _split_sync_waits(nc)
    return nc


@functools.lru_cache(maxsize=2)
def _built(s):
    return build_kernel(s)


def _rope_tables(s):
    j = np.arange(D // 2, dtype=np.float64)
    inv = THETA ** (-2.0 * j / D)
    ang = np.arange(s, dtype=np.float64)[:, None] * inv[None, :]  # [s, 32]
    cos = np.cos(ang).T  # [32, s]
    sin = np.sin(ang).T
    cos64 = np.repeat(cos, 2, axis=0)  # rows 2j, 2j+1 identical
    sin64 = np.repeat(sin, 2, axis=0)
    # "pre-swap" sign convention: the kernel multiplies by this table BEFORE
    # pair-swapping partitions, so odd rows carry the minus sign.
    sin64[1::2, :] *= -1.0
    cos128 = np.concatenate([cos64, cos64], axis=0).astype(np.float32)
    sin128 = np.concatenate([sin64, sin64], axis=0).astype(np.float32)
    return np.ascontiguousarray(cos128), np.ascontiguousarray(sin128)


def _col_tiled(vec, tile_rows=128):
    """[n] -> [tile_rows, n//tile_rows], column t = vec[t*128:(t+1)*128]."""
    n = vec.shape[0]
    return np.ascontiguousarray(vec.reshape(n // tile_rows, tile_rows).T)


def _bf16(a):
    import ml_dtypes

    return np.ascontiguousarray(np.asarray(a).astype(ml_dtypes.bfloat16))


def make_in_maps(x, attention_padding, W_qkv, b_qkv, W_proj, b_proj):
    x = np.asarray(x, dtype=np.float32)
    pad = np.asarray(attention_padding).astype(bool)
    W_qkv = np.asarray(W_qkv, dtype=np.float32)
    b_qkv = np.asarray(b_qkv, dtype=np.float32)
    W_proj = np.asarray(W_proj, dtype=np.float32)
    b_proj = np.asarray(b_proj, dtype=np.float32)
    B, s, _ = x.shape
    nkt = s // 128
    cos128, sin128 = _rope_tables(s)

    per_hp = {}
    for hp in range(2):
        hs = slice(hp * NHL * D, (hp + 1) * NHL * D)
        Wq = W_qkv[:, 0:HID][:, hs]
        Wk = W_qkv[:, HID : 2 * HID][:, hs]
        Wv = W_qkv[:, 2 * HID : 3 * HID][:, hs]
        bq = b_qkv[0:HID][hs]
        bk = b_qkv[HID : 2 * HID][hs]
        bv = b_qkv[2 * HID : 3 * HID][hs]
        bqk = np.concatenate([bq, bk])
        per_hp[hp] = dict(
            w_qk=_bf16(np.concatenate([Wq, Wk], axis=1)),
            w_v=_bf16(Wv),
            w_proj=_bf16(W_proj[hs, :]),
            b_qk=_col_tiled(bqk),
            b_v=_bf16(bv[None, :]),
            b_proj=_bf16((b_proj if hp == 0 else np.zeros_like(b_proj))[None, :]),
        )

    per_b = {}
    for b in range(B):
        p = pad[b].astype(np.float32)
        per_b[b] = dict(
            xt=_bf16(x[b].T),
            pad_mul=_col_tiled(p),
            pad_vcol=_bf16(np.repeat(_col_tiled(p), NHL, axis=1)),
        )

    in_maps = []
    for c in range(2 * B):
        b, hp = c // 2, c % 2
        m = dict(per_hp[hp])
        m.update(per_b[b])
        m["cos_t"] = cos128
        m["sin_t"] = sin128
        m["ones_b"] = _bf16(np.ones((1, 128)))
        j = np.arange(128)
        tri = (j[None, :] >= j[:, None]).astype(np.float32)  # keep q >= k
        m["tri_m"] = _bf16(np.tile(tri, (1, 2)))
        in_maps.append(m)
    return in_maps


def run(x, attention_padding, W_qkv, b_qkv, W_proj, b_proj, trace=False, **spmd_kw):
    x = np.asarray(x, dtype=np.float32)
    B, s, _ = x.shape
    nc = _built(s)
    in_maps = make_in_maps(x, attention_padding, W_qkv, b_qkv, W_proj, b_proj)
    res = run_bass_kernel_spmd(nc, in_maps, list(range(2 * B)), trace=trace, **spmd_kw)
    out = np.stack(
        [res.results[2 * b]["z"] + res.results[2 * b + 1]["z"] for b in range(B)]
    ).astype(np.float32)
    return out, res


def kernel(x, attention_padding, W_qkv, b_qkv, W_proj, b_proj, train=None, **_):
    out, _res = run(x, attention_padding, W_qkv, b_qkv, W_proj, b_proj)
    return out
